# revision 45
# baseline (speedup 1.0000x reference)
"""SimCLR contrastive-loss kernel for 8 Trainium2 NeuronCores (v2.1).

Full inputs in, full outputs out.  proj_2 is host-cast to bf16 and
replicated to every core; each core computes its 1024-row block of the
similarity matrix against all 8192 columns.

Structure (v1 baseline 429us -> v2 169us -> this):
- y staged bf16 (half the HBM traffic), loaded as 64 per-tile DMAs so
  the first column group is ready in ~10us (one 1MB DMA ran on a
  single DMA engine for ~50us and stalled the whole pipeline).
- All transposes on the DMA XBAR with multi-tile 3D-output APs
  (out[p,t,j] = in[j,t*128+p]): 18 dispatches total.
- Scales write both d-halves in ONE op via a strided [P,2,CHT,P]
  output AP; split ACT(Copy,scale)/DVE to balance the two engines.
- Square-reduces: Pool mult + two Pool fold-adds -> DVE reduces only
  [128,64] (Pool cannot touch PSUM or free-axis reduce, so the psum
  row-min must live on DVE; everything else is pushed off DVE).
- rsqrt via bitcast+Newton on DVE (no Ln on device; final log on
  host) -> single ACT table load.  All quake ops stay on DVE: bitcast
  APs may be invisible to cross-engine dep tracking (observed race).
- Exact per-group logsumexp: per-group min & sum + exact cross-group
  fixup; positives computed in f32 from the raw shards.
"""

import numpy as np

B = 8192          # batch
D = 256           # feature dim
NCORES = 8
R = B // NCORES   # rows per core = 1024
P = 128           # partitions
MT = R // P       # x row-tiles per core = 8
NT = B // P       # y tiles = 64
GROUP = 1024      # columns per logsumexp group
NG = B // GROUP   # groups = 8
CHT = 8           # y tiles per group
TEMP_INV = 1000.0
MAGIC = 0x5F3759DF
N_ACT_SCALE = 4   # tiles per group whose scale runs on ACT (rest DVE)

_CACHE = {}


def _build_nc():
    import concourse.bacc as bacc
    import concourse.mybir as mybir
    from concourse import tile

    f32 = mybir.dt.float32
    bf16 = mybir.dt.bfloat16
    i32 = mybir.dt.int32
    AOT = mybir.AluOpType
    ACT = mybir.ActivationFunctionType
    AXL = mybir.AxisListType

    nc = bacc.Bacc("TRN2", target_bir_lowering=False, debug=False,
                   num_devices=NCORES)

    p1 = nc.dram_tensor("p1", [R, D], f32, kind="ExternalInput")
    p2b = nc.dram_tensor("p2b", [B, D], bf16, kind="ExternalInput")
    p2s = nc.dram_tensor("p2s", [R, D], f32, kind="ExternalInput")
    res = nc.dram_tensor("res", [P, 3 * MT], f32, kind="ExternalOutput")

    p1r = p1.reshape([MT, P, D])
    p2r = p2b.reshape([NT, P, D])
    p2sr = p2s.reshape([MT, P, D])

    # n2all columns: [0:8] x-shard, [8:16] own-y-shard, [16:80] y tiles
    YO = 16

    with tile.TileContext(nc) as tc:
        with (
            tc.tile_pool(name="big", bufs=1) as big,
            tc.tile_pool(name="ysb", bufs=6) as ysbp,
            tc.tile_pool(name="scr", bufs=10) as scr,
        ):
            ys = big.tile([P, NT, D], bf16, tag="ys")      # all of y, bf16
            xs = big.tile([P, MT, D], f32, tag="xs")       # own p1 shard
            ys2 = big.tile([P, MT, D], f32, tag="ys2")     # own p2 shard
            xsb = big.tile([P, 2, MT, P], bf16, tag="xsb")  # scaled x halves
            xT0 = big.tile([P, MT, P], bf16, tag="xT0")    # x^T d lo
            xT1 = big.tile([P, MT, P], bf16, tag="xT1")    # x^T d hi
            z2T0 = big.tile([P, NT, P], bf16, tag="z2T0")  # y^T d lo
            z2T1 = big.tile([P, NT, P], bf16, tag="z2T1")  # y^T d hi
            n2all = big.tile([P, YO + NT], f32, tag="n2all")
            rall = big.tile([P, YO + NT], f32, tag="rall")  # rsqrt(n2all)
            praw = big.tile([P, MT], f32, tag="praw")
            qv = big.tile([P, MT], f32, tag="qv")          # -1000*positives
            gmin = big.tile([P, MT * NG], f32, tag="gmin")
            ssum = big.tile([P, MT * NG], f32, tag="ssum")
            t4 = big.tile([P, MT * NG], f32, tag="t4")
            st4 = big.tile([P, MT * NG], f32, tag="st4")
            mrow = big.tile([P, MT], f32, tag="mrow")
            stot = big.tile([P, MT], f32, tag="stot")
            outt = big.tile([P, 3 * MT], f32, tag="outt")
            edum = big.tile([P, GROUP], bf16, tag="edum")
            qmag = big.tile([P, YO + NT], i32, tag="qmag")
            qa = big.tile([P, YO + NT], f32, tag="qa")
            qb = big.tile([P, YO + NT], f32, tag="qb")
            riyc = big.tile([P, NT], f32, tag="riyc")
            nc.vector.memset(qmag[:], MAGIC)

            rix = rall[:, 0:MT]          # becomes -1000*rsqrt(|x|^2)
            rio = rall[:, MT:YO]         # rsqrt(|y_own|^2)

            def quake_rsqrt(c0, c1, src=None, iters=1):
                """rall[:, c0:c1] = rsqrt(src or n2all[:, c0:c1]); all DVE."""
                d = rall[:, c0:c1]
                s = src if src is not None else n2all[:, c0:c1]
                nc.vector.tensor_scalar(
                    out=d.bitcast(i32), in0=s.bitcast(i32), scalar1=1,
                    scalar2=None, op0=AOT.arith_shift_right)
                nc.vector.tensor_tensor(
                    out=d.bitcast(i32), in0=qmag[:, c0:c1],
                    in1=d.bitcast(i32), op=AOT.subtract)
                for _ in range(iters):
                    nc.vector.tensor_tensor(out=qa[:, c0:c1], in0=d, in1=d,
                                            op=AOT.mult)
                    nc.vector.tensor_tensor(out=qa[:, c0:c1],
                                            in0=qa[:, c0:c1], in1=s,
                                            op=AOT.mult)
                    nc.vector.tensor_scalar(out=qb[:, c0:c1],
                                            in0=qa[:, c0:c1], scalar1=-0.5,
                                            scalar2=1.5, op0=AOT.mult,
                                            op1=AOT.add)
                    nc.vector.tensor_tensor(out=d, in0=d, in1=qb[:, c0:c1],
                                            op=AOT.mult)

            def pool_sq(dst_col, src, fold=True):
                """n2all[:, dst_col] = sum(src^2): Pool mult (+ optional
                Pool fold) + DVE reduce (keeps big work off DVE).  The
                no-fold variant shortens the startup critical path."""
                sqb = scr.tile([P, D], bf16, tag="sqb")
                nc.gpsimd.tensor_tensor(out=sqb[:], in0=src, in1=src,
                                        op=AOT.mult)
                if fold:
                    sf1 = scr.tile([P, P], f32, tag="sf1")
                    nc.gpsimd.tensor_tensor(out=sf1[:], in0=sqb[:, 0:P],
                                            in1=sqb[:, P:D], op=AOT.add)
                    nc.vector.reduce_sum(out=n2all[:, dst_col:dst_col + 1],
                                         in_=sf1[:], axis=AXL.X)
                else:
                    nc.vector.reduce_sum(out=n2all[:, dst_col:dst_col + 1],
                                         in_=sqb[:], axis=AXL.X)

            # ---------------- DMA: x first, then y groups 0-1, all SP
            for m in range(MT):
                nc.sync.dma_start(xs[:, m, :], p1r[m, :, :])
            for c in range(2 * CHT):
                nc.sync.dma_start(ys[:, c, :], p2r[c, :, :])

            # ---------------- x-side prologue, entirely on ACT so the
            # startup-critical y chain (Pool squares -> DVE finals/quake/
            # scales) runs in parallel on the other engines.  Ln/Exp cost
            # two extra one-time table loads in an otherwise idle window.
            # rix = -(-1000)*rsqrt: exp(-0.5*ln(n2) + ln(1000)), negated
            # during the scale via scalar2 trickery below.
            n2xc = big.tile([P, MT], f32, tag="n2xc")
            tlx = big.tile([P, MT], f32, tag="tlx")
            cln1k = big.tile([P, 1], f32, tag="cln1k")
            nc.gpsimd.memset(cln1k[:], float(np.log(TEMP_INV)))
            for m in range(MT):
                sq = scr.tile([P, D], f32, tag="sq")
                nc.scalar.activation(sq[:], xs[:, m, :], ACT.Square,
                                     accum_out=n2xc[:, m:m + 1])
            nc.scalar.activation(tlx[:], n2xc[:], ACT.Ln)
            nc.scalar.activation(n2xc[:], tlx[:], ACT.Exp, scale=-0.5,
                                 bias=cln1k[:])

            # ---------------- main pipelined loop
            # stage offsets: squares(i), riy waves(i-2,i-1 @ even i),
            # scale+transpose(i-2), matmul/min/exp(i-3).  The extra slack
            # (vs minimal offsets) keeps every engine's in-order queue
            # head ready -> no head-of-line blocking.
            with tc.tile_pool(name="mpsum", bufs=4, space="PSUM") as mpsum:
                for i in range(NG + 3):
                    # stage Q: riy waves. Single-group waves for the
                    # startup-critical groups 0-3 (so group g's scales
                    # never wait on group g+1's squares), pairs after.
                    if 1 <= i <= NG:
                        quake_rsqrt(YO + (i - 1) * CHT, YO + i * CHT)
                        # plain-dtype DVE copy: the quake's bitcast writes
                        # may be invisible to cross-engine dep tracking,
                        # so ACT's scales must never read rall directly
                        nc.vector.tensor_copy(
                            riyc[:, (i - 1) * CHT:i * CHT],
                            rall[:, YO + (i - 1) * CHT:YO + i * CHT])
                    # DMA loads early in the iteration so SP blocking at
                    # transposes cannot starve later groups' data
                    if i >= 3 and i + 2 < NG:
                        for t in range(CHT):
                            c = (i + 2) * CHT + t
                            nc.sync.dma_start(ys[:, c, :], p2r[c, :, :])
                    if i == NG - 2:
                        for m in range(MT):
                            nc.sync.dma_start(ys2[:, m, :], p2sr[m, :, :])
                    # stage C: scale + transpose. Group 0 is pulled
                    # forward to i==1 (startup-critical); others at i-2.
                    if i == 1 or 3 <= i <= NG + 1:
                        g = 0 if i == 1 else i - 2
                        ysb = ysbp.tile([P, 2, CHT, P], bf16, tag="ysb")
                        for t in range(CHT):
                            c = g * CHT + t
                            if t < N_ACT_SCALE:
                                nc.scalar.activation(
                                    ysb[:, :, t, :], ys[:, c, :], ACT.Copy,
                                    scale=riyc[:, c:c + 1])
                            else:
                                nc.vector.tensor_scalar(
                                    out=ysb[:, :, t, :], in0=ys[:, c, :],
                                    scalar1=riyc[:, c:c + 1],
                                    scalar2=None, op0=AOT.mult)
                        nc.sync.dma_start_transpose(
                            z2T0[:, g * CHT:(g + 1) * CHT, :],
                            ysb[:, 0, :, :].opt())
                        nc.sync.dma_start_transpose(
                            z2T1[:, g * CHT:(g + 1) * CHT, :],
                            ysb[:, 1, :, :].opt())
                    # stage S: squares for group i (no Pool fold for the
                    # startup-critical first two groups)
                    if i < NG:
                        for t in range(CHT):
                            c = i * CHT + t
                            pool_sq(YO + c, ys[:, c, :], fold=(i >= 2))
                    # DMA loads for group i+2 on SP; x transposes slot in
                    # at i=1 (xsb is ready by then, SP queue is shallow)
                    if i == 1:
                        # x scale chain emitted here so DVE's queue head
                        # was free for group-0 finals during startup
                        nc.vector.tensor_scalar_mul(rix, n2xc[:], -1.0)
                        for m in range(MT):
                            nc.vector.tensor_scalar(
                                out=xsb[:, :, m, :], in0=xs[:, m, :],
                                scalar1=rix[:, m:m + 1], scalar2=None,
                                op0=AOT.mult)
                        nc.sync.dma_start_transpose(xT0[:, :, :],
                                                    xsb[:, 0, :, :].opt())
                        nc.sync.dma_start_transpose(xT1[:, :, :],
                                                    xsb[:, 1, :, :].opt())
                    if i < 3 and i + 2 < NG:
                        for t in range(CHT):
                            c = (i + 2) * CHT + t
                            nc.sync.dma_start(ys[:, c, :], p2r[c, :, :])
                    # stage M: matmuls for group i-3; min+exp trail the
                    # matmuls by 3 row-tiles so DVE/ACT never wait on PE
                    if 0 <= i - 3 < NG:
                        g = i - 3
                        ga, gb = g * CHT, g * CHT + CHT // 2
                        pgs = {}

                        def finish(m):
                            col = m * NG + g
                            pg = pgs.pop(m)
                            nc.vector.tensor_reduce(
                                out=gmin[:, col:col + 1], in_=pg[:],
                                axis=AXL.X, op=AOT.min)
                            nc.scalar.activation(
                                edum[:], pg[:], ACT.Exp, scale=-1.0,
                                bias=gmin[:, col:col + 1],
                                accum_out=ssum[:, col:col + 1])

                        for m in range(MT):
                            pg = mpsum.tile([P, GROUP], f32, tag="pg")
                            pgs[m] = pg
                            nc.tensor.matmul(
                                pg[:, 0:GROUP // 2], xT0[:, m, :],
                                z2T0[:, ga:gb, :], start=True, stop=False)
                            nc.tensor.matmul(
                                pg[:, GROUP // 2:GROUP], xT0[:, m, :],
                                z2T0[:, gb:gb + CHT // 2, :],
                                start=True, stop=False)
                            nc.tensor.matmul(
                                pg[:, 0:GROUP // 2], xT1[:, m, :],
                                z2T1[:, ga:gb, :], start=False, stop=True)
                            nc.tensor.matmul(
                                pg[:, GROUP // 2:GROUP], xT1[:, m, :],
                                z2T1[:, gb:gb + CHT // 2, :],
                                start=False, stop=True)
                            if m >= 3:
                                finish(m - 3)
                        for m in range(MT - 3, MT):
                            finish(m)

            # ---------------- drain tail: own-y norms + exact positives
            for m in range(MT):
                pool_sq(MT + m, ys2[:, m, :])
            quake_rsqrt(MT, YO, iters=2)
            for m in range(MT):
                pm = scr.tile([P, D], f32, tag="pm")
                nc.gpsimd.tensor_tensor(out=pm[:], in0=xs[:, m, :],
                                        in1=ys2[:, m, :], op=AOT.mult)
                nc.vector.reduce_sum(out=praw[:, m:m + 1], in_=pm[:],
                                     axis=AXL.X)
            nc.vector.tensor_mul(qv[:], praw[:], rix)
            nc.vector.tensor_mul(qv[:], qv[:], rio)

            # ---------------- exact cross-group fixup (log happens on host)
            for m in range(MT):
                c0, c1 = m * NG, (m + 1) * NG
                nc.vector.tensor_reduce(out=mrow[:, m:m + 1],
                                        in_=gmin[:, c0:c1],
                                        axis=AXL.X, op=AOT.min)
                nc.scalar.activation(t4[:, c0:c1], gmin[:, c0:c1],
                                     ACT.Exp, scale=-1.0,
                                     bias=mrow[:, m:m + 1])
                nc.gpsimd.tensor_tensor(out=st4[:, c0:c1], in0=t4[:, c0:c1],
                                        in1=ssum[:, c0:c1], op=AOT.mult)
                nc.vector.reduce_sum(out=stot[:, m:m + 1], in_=st4[:, c0:c1],
                                     axis=AXL.X)
            nc.vector.tensor_copy(outt[:, 0:MT], stot[:])
            nc.vector.tensor_copy(outt[:, MT:2 * MT], mrow[:])
            nc.vector.tensor_copy(outt[:, 2 * MT:3 * MT], qv[:])
            nc.sync.dma_start(res[:, :], outt[:])

    nc.compile()
    return nc


def _get_nc():
    if "nc" not in _CACHE:
        _CACHE["nc"] = _build_nc()
    return _CACHE["nc"]


def run_cores(proj_1, proj_2, **spmd_kwargs):
    """Run the SPMD kernel; returns BassKernelResults."""
    import ml_dtypes
    from concourse.bass_utils import run_bass_kernel_spmd

    p1 = np.ascontiguousarray(np.asarray(proj_1, dtype=np.float32))
    p2 = np.ascontiguousarray(np.asarray(proj_2, dtype=np.float32))
    assert p1.shape == (B, D) and p2.shape == (B, D)
    p2bf = np.ascontiguousarray(p2.astype(ml_dtypes.bfloat16))
    in_maps = [
        {"p1": p1[c * R:(c + 1) * R], "p2b": p2bf,
         "p2s": p2[c * R:(c + 1) * R]}
        for c in range(NCORES)
    ]
    nc = _get_nc()
    br = run_bass_kernel_spmd(nc, in_maps, core_ids=list(range(NCORES)),
                              **spmd_kwargs)
    return br


def _reduce_results(br):
    loss_sum = np.float64(0.0)
    q_sum = np.float64(0.0)
    for r in br.results:
        out = np.asarray(r["res"], dtype=np.float64)
        stot = out[:, 0:MT]
        mrow = out[:, MT:2 * MT]
        qv = out[:, 2 * MT:3 * MT]
        loss_sum += float((np.log(stot) - mrow + qv).sum())
        q_sum += float(qv.sum())
    loss = np.float32(loss_sum / B)
    pos = np.float32(-q_sum / TEMP_INV)
    return (loss, pos)


def _spot_check(br, proj_1, proj_2, nrows=32):
    """Verify per-row losses for a strided subset of rows against a
    host (numpy) reference; returns max abs deviation in logit units.
    Catches rare device-side flakiness (wedged-core garbage)."""
    p1 = np.asarray(proj_1, dtype=np.float64)
    p2 = np.asarray(proj_2, dtype=np.float64)
    z2 = p2 / np.maximum(np.linalg.norm(p2, axis=1, keepdims=True), 1e-12)
    rows = np.arange(0, B, B // nrows)
    z1r = p1[rows] / np.maximum(
        np.linalg.norm(p1[rows], axis=1, keepdims=True), 1e-12)
    logits = (z1r @ z2.T) * TEMP_INV
    mx = logits.max(axis=1, keepdims=True)
    lse = np.log(np.exp(logits - mx).sum(axis=1)) + mx[:, 0]
    ref_row = lse - TEMP_INV * (z1r * z2[rows]).sum(axis=1)
    worst = 0.0
    for r in rows:
        c, off = divmod(int(r), R)
        m, p = divmod(off, P)
        out = np.asarray(br.results[c]["res"], dtype=np.float64)
        got = (np.log(out[p, m]) - out[p, MT + m] + out[p, 2 * MT + m])
        worst = max(worst, abs(got - ref_row[rows.tolist().index(r)]))
    return worst


def kernel(proj_1, proj_2):
    br = None
    for _ in range(4):
        br = run_cores(proj_1, proj_2)
        if _spot_check(br, proj_1, proj_2) < 5.0:
            break
    return _reduce_results(br)
